# revision 3
# baseline (speedup 1.0000x reference)
"""Fused MoE + per-expert LoRA, expert-parallel across 8 TRN2 NeuronCores.

Strategy (sharding hint: expert-parallel):
  - Host dispatches the T*K routed (token, expert) pairs: core e gets the
    tokens routed to expert e, gathered + transposed to [H, C] (C = padded
    max per-expert count), plus expert e's w13/w2 and the 4 LoRA adapters'
    A/B for expert e, all pre-transposed on host into the exact SBUF
    layouts the kernel consumes (contiguous DMAs).
  - LoRA is fused into the base GEMMs: the 4 adapters' A matrices are
    concatenated to [64, H] so one extra K=128-tile matmul chain produces
    all mid-products; a [64, C] mask (scaling[l] where the pair's adapter
    == l, else 0) selects the right adapter per column; the 4 B matrices
    concatenated to [64, N] accumulate into the same PSUM tile as the base
    GEMM (one extra K=64 matmul per output tile).
  - Routing weight is folded into the activation (act = silu(gate)*up*w),
    which makes both the base down-GEMM and the down-LoRA delta carry it.
  - Host scatter-adds each core's [H, C] output back over the top_k axis.
  - Matmuls run in float32r (full-rate fp32 streaming, ~1e-4 rel err).
"""

import numpy as np
from contextlib import ExitStack

import concourse.bass as bass
import concourse.tile as tile
from concourse import bacc, mybir
from concourse.bass_utils import run_bass_kernel_spmd

T, H, I, E, K, L, R = 1024, 1024, 1024, 8, 2, 4, 16
N = 2 * I
P = 128
KH = H // P    # k-tiles over H (gate_up contraction)
KI = I // P    # k-tiles over I (down contraction)
NT = N // P    # n-tiles of gate_up output
HT = H // P    # h-tiles of down output
LR = L * R     # concatenated lora rank

W13_GRP = 4              # n-tile slots per DMA group
GS13 = KH * W13_GRP * P  # free-dim span of one w13 group in SBUF
W2_GRP = 4
GS2 = KI * W2_GRP * P
# w13 SBUF slot s holds n-tile W13_PERM[s]: gate/up tiles interleaved so the
# compute loop consumes slots sequentially (pair j = slots 2j, 2j+1)
W13_PERM = [t for j in range(NT // 2) for t in (j, j + NT // 2)]

_CACHE: dict = {}

# f16 halves HBM traffic vs f32 (w13+w2 dominate); matmul streams at the
# same 1 col/cycle either way, so this moves the kernel from DMA-bound
# (~46us of weight DMA) to PE-bound (~29us). randn-scale data sits well
# inside f16 range; rel err ~1e-3 << the 2e-2 gate.
MODE = "f16"


def _round_up(x, m):
    return ((x + m - 1) // m) * m


def _np_dt(mode):
    if mode == "bf16":
        import ml_dtypes
        return np.dtype(ml_dtypes.bfloat16)
    if mode == "f16":
        return np.dtype(np.float16)
    return np.dtype(np.float32)


def _mm_dt(mode):
    return {"f32": mybir.dt.float32,
            "f32r": mybir.dt.float32r,
            "bf16": mybir.dt.bfloat16,
            "f16": mybir.dt.float16}[mode]


def _prep_in_maps(hidden_states, topk_weights, w13, w2, gate_up_lora_a,
                  gate_up_lora_b, down_lora_a, down_lora_b, scalings,
                  topk_ids, lora_indices, mode=None):
    """Host-side dispatch: returns (in_maps, idx_per_expert, tok, C)."""
    ndt = _np_dt(mode)
    hidden_states = np.asarray(hidden_states, dtype=np.float32)
    topk_weights = np.asarray(topk_weights, dtype=np.float32)
    w13 = np.asarray(w13, dtype=np.float32)
    w2 = np.asarray(w2, dtype=np.float32)
    gua = np.asarray(gate_up_lora_a, dtype=np.float32)
    gub = np.asarray(gate_up_lora_b, dtype=np.float32)
    dla = np.asarray(down_lora_a, dtype=np.float32)
    dlb = np.asarray(down_lora_b, dtype=np.float32)
    scalings = np.asarray(scalings, dtype=np.float32)
    topk_ids = np.asarray(topk_ids)
    lora_indices = np.asarray(lora_indices)

    e_flat = topk_ids.reshape(-1).astype(np.int64)
    l_flat = np.repeat(lora_indices, K).astype(np.int64)
    w_flat = topk_weights.reshape(-1).astype(np.float32)
    tok = np.arange(T * K) // K

    idx_per = [np.nonzero(e_flat == e)[0] for e in range(E)]
    maxc = max(1, max(len(ix) for ix in idx_per))
    C = max(128, _round_up(maxc, 32))

    in_maps = []
    for e in range(E):
        ix = idx_per[e]
        cnt = len(ix)

        xg = np.zeros((C, H), np.float32)
        xg[:cnt] = hidden_states[tok[ix]]
        xt = np.ascontiguousarray(
            xg.T.reshape(KH, P, C).transpose(1, 0, 2).reshape(P, KH * C)).astype(ndt)

        w13t = w13[e].T  # [H, N]
        # slot order interleaves gate/up n-tiles: slot 2j = gate j, 2j+1 = up j
        w13_tiles = w13t.reshape(KH, P, NT, P)[:, :, W13_PERM]
        w13_hbm = np.ascontiguousarray(
            w13_tiles.reshape(KH, P, NT // W13_GRP, W13_GRP * P)
            .transpose(1, 2, 0, 3).reshape(P, KH * N)).astype(ndt)

        w2t = w2[e].T  # [I, H]
        w2_hbm = np.ascontiguousarray(
            w2t.reshape(KI, P, H // (W2_GRP * P), W2_GRP * P)
            .transpose(1, 2, 0, 3).reshape(P, KI * H)).astype(ndt)

        ag = np.ascontiguousarray(
            gua[:, e].reshape(LR, H).T.reshape(KH, P, LR)
            .transpose(1, 0, 2).reshape(P, KH * LR)).astype(ndt)
        ad = np.ascontiguousarray(
            dla[:, e].reshape(LR, I).T.reshape(KI, P, LR)
            .transpose(1, 0, 2).reshape(P, KI * LR)).astype(ndt)
        bg = np.ascontiguousarray(
            gub[:, e].transpose(0, 2, 1).reshape(LR, N)).astype(ndt)
        bd = np.ascontiguousarray(
            dlb[:, e].transpose(0, 2, 1).reshape(LR, H)).astype(ndt)

        msk = np.zeros((LR, C), np.float32)
        if cnt:
            lv = l_flat[ix]
            m_small = (lv[None, :] == np.arange(L)[:, None]) * scalings[:, None]
            msk[:, :cnt] = np.repeat(m_small.astype(np.float32), R, axis=0)

        wv = np.zeros((P, C), np.float32)
        if cnt:
            wv[:, :cnt] = w_flat[ix][None, :]

        in_maps.append({
            "xt": xt, "w13t": w13_hbm, "w2t": w2_hbm,
            "agt": ag, "adt": ad, "bgt": bg, "bdt": bd,
            "msk": msk, "wv": wv,
        })
    return in_maps, idx_per, tok, C


def _combine(results, idx_per, tok, C):
    out = np.zeros((T, H), np.float32)
    for e in range(E):
        ix = idx_per[e]
        cnt = len(ix)
        if cnt == 0:
            continue
        outt = results[e]["outt"].reshape(P, HT, C).transpose(1, 0, 2).reshape(H, C)
        np.add.at(out, tok[ix], outt[:, :cnt].T)
    return out


def _build(C, mode=None, repeat=1, loop_reps=0, body="full"):
    """Trace + compile the per-core bass program for padded count C.

    loop_reps > 0 wraps the body in a device-side For_i loop (timing only).
    body: "full" | "dma" (loads/stores only, no compute) | "compute"
    (loads hoisted out of the timing loop) - diagnostics only.
    """
    f32 = mybir.dt.float32
    mdt = _mm_dt(mode)
    nc = bacc.Bacc("TRN2", target_bir_lowering=False, debug=False, num_devices=E)

    xt_d = nc.declare_dram_parameter("xt", [P, KH * C], mdt, isOutput=False)
    w13_d = nc.declare_dram_parameter("w13t", [P, KH * N], mdt, isOutput=False)
    w2_d = nc.declare_dram_parameter("w2t", [P, KI * H], mdt, isOutput=False)
    ag_d = nc.declare_dram_parameter("agt", [P, KH * LR], mdt, isOutput=False)
    ad_d = nc.declare_dram_parameter("adt", [P, KI * LR], mdt, isOutput=False)
    bg_d = nc.declare_dram_parameter("bgt", [LR, N], mdt, isOutput=False)
    bd_d = nc.declare_dram_parameter("bdt", [LR, H], mdt, isOutput=False)
    msk_d = nc.declare_dram_parameter("msk", [LR, C], f32, isOutput=False)
    wv_d = nc.declare_dram_parameter("wv", [P, C], f32, isOutput=False)
    out_d = nc.declare_dram_parameter("outt", [P, HT * C], f32, isOutput=True)

    # column blocks of at most 512 (PSUM free-dim limit for fp32)
    n_blk = (C + 511) // 512
    step = (C + n_blk - 1) // n_blk
    blks = [(b * step, min(C, (b + 1) * step)) for b in range(n_blk)]

    silu_fn = mybir.ActivationFunctionType.Silu

    with tile.TileContext(nc) as tc:
        with ExitStack() as ctx:
            static = ctx.enter_context(tc.tile_pool(name="static", bufs=1))
            work = ctx.enter_context(tc.tile_pool(name="work", bufs=4))
            ptmp = ctx.enter_context(tc.tile_pool(name="ptmp", bufs=1, space="PSUM"))
            pc1 = ctx.enter_context(tc.tile_pool(name="pc1", bufs=2, space="PSUM"))
            pc3 = ctx.enter_context(tc.tile_pool(name="pc3", bufs=2, space="PSUM"))

            # double-buffer input tiles for 2-byte modes so a loop/repeat
            # iteration's DMAs overlap the previous iteration's compute
            # (f32 tiles are too big to double-buffer in SBUF)
            sb = 2 if mybir.dt.size(mdt) == 2 else 1

            def make_tiles():
                t = {}
                t["xt"] = static.tile([P, KH * C], mdt, tag="xt", name="xt_sb", bufs=sb)
                t["w13"] = static.tile([P, KH * N], mdt, tag="w13", name="w13_sb", bufs=sb)
                t["w2"] = static.tile([P, KI * H], mdt, tag="w2", name="w2_sb", bufs=sb)
                t["ag"] = static.tile([P, KH * LR], mdt, tag="ag", name="ag_sb", bufs=sb)
                t["ad"] = static.tile([P, KI * LR], mdt, tag="ad", name="ad_sb", bufs=sb)
                t["bg"] = static.tile([LR, N], mdt, tag="bg", name="bg_sb", bufs=sb)
                t["bd"] = static.tile([LR, H], mdt, tag="bd", name="bd_sb", bufs=sb)
                t["msk"] = static.tile([LR, C], f32, tag="msk", name="msk_sb", bufs=sb)
                t["wv"] = static.tile([P, C], f32, tag="wv", name="wv_sb", bufs=sb)
                t["act"] = static.tile([P, KI * C], mdt, tag="act", name="act_sb")
                t["out"] = static.tile([P, HT * C], f32, tag="out", name="out_sb")
                t["xlg"] = static.tile([LR, C], mdt, tag="xlg", name="xlg_sb")
                t["xld"] = static.tile([LR, C], mdt, tag="xld", name="xld_sb")
                return t

            def emit_loads(t):
                nc.sync.dma_start(t["xt"][:], xt_d[:])
                nc.sync.dma_start(t["ag"][:], ag_d[:])
                nc.sync.dma_start(t["msk"][:], msk_d[:])
                nc.scalar.dma_start(t["bg"][:], bg_d[:])
                nc.scalar.dma_start(t["wv"][:], wv_d[:])
                # w13 groups alternate rings, in consumption order
                for g in range(N // (W13_GRP * P)):
                    eng = nc.sync if g % 2 == 0 else nc.scalar
                    eng.dma_start(t["w13"][:, g * GS13:(g + 1) * GS13],
                                  w13_d[:, g * GS13:(g + 1) * GS13])
                # phase-B tensors after w13
                nc.sync.dma_start(t["ad"][:], ad_d[:])
                nc.scalar.dma_start(t["bd"][:], bd_d[:])
                for g in range(H // (W2_GRP * P)):
                    eng = nc.sync if g % 2 == 0 else nc.scalar
                    eng.dma_start(t["w2"][:, g * GS2:(g + 1) * GS2],
                                  w2_d[:, g * GS2:(g + 1) * GS2])

            def emit_compute(t):
                for (c0, c1) in blks:
                    cw = c1 - c0

                    def xts(kt):
                        return t["xt"][:, kt * C + c0: kt * C + c1]

                    def acts(kt):
                        return t["act"][:, kt * C + c0: kt * C + c1]

                    # gate_up lora mid-product
                    tmp_g = ptmp.tile([LR, cw], f32, tag="tmpg")
                    for kt in range(KH):
                        nc.tensor.matmul(
                            tmp_g[:], t["ag"][:, kt * LR:(kt + 1) * LR], xts(kt),
                            start=(kt == 0), stop=(kt == KH - 1))
                    nc.vector.tensor_mul(t["xlg"][:, c0:c1], tmp_g[:],
                                         t["msk"][:, c0:c1])

                    # gate_up GEMM + lora, act = silu(gate)*up*w
                    for j in range(KI):
                        pair = []
                        for jn in (j, j + KI):
                            c1t = pc1.tile([P, cw], f32,
                                           tag="c1g" if jn == j else "c1u")
                            slot = 2 * j + (0 if jn == j else 1)
                            g, jj = divmod(slot, W13_GRP)
                            for kt in range(KH):
                                off = g * GS13 + kt * (W13_GRP * P) + jj * P
                                nc.tensor.matmul(
                                    c1t[:], t["w13"][:, off:off + P], xts(kt),
                                    start=(kt == 0), stop=False)
                            nc.tensor.matmul(
                                c1t[:], t["bg"][:, jn * P:(jn + 1) * P],
                                t["xlg"][:, c0:c1], start=False, stop=True)
                            pair.append(c1t)
                        c1g, c1u = pair
                        silu_t = work.tile([P, cw], f32, tag="silu")
                        up_w = work.tile([P, cw], f32, tag="upw")
                        nc.scalar.activation(silu_t[:], c1g[:], silu_fn)
                        nc.vector.tensor_mul(up_w[:], c1u[:], t["wv"][:, c0:c1])
                        nc.vector.tensor_mul(acts(j), silu_t[:], up_w[:])

                    # down lora mid-product
                    tmp_d = ptmp.tile([LR, cw], f32, tag="tmpd")
                    for kt in range(KI):
                        nc.tensor.matmul(
                            tmp_d[:], t["ad"][:, kt * LR:(kt + 1) * LR], acts(kt),
                            start=(kt == 0), stop=(kt == KI - 1))
                    nc.vector.tensor_mul(t["xld"][:, c0:c1], tmp_d[:],
                                         t["msk"][:, c0:c1])

                    # down GEMM + lora
                    for h in range(HT):
                        c3t = pc3.tile([P, cw], f32, tag="c3")
                        g, jj = divmod(h, W2_GRP)
                        for kt in range(KI):
                            off = g * GS2 + kt * (W2_GRP * P) + jj * P
                            nc.tensor.matmul(
                                c3t[:], t["w2"][:, off:off + P], acts(kt),
                                start=(kt == 0), stop=False)
                        nc.tensor.matmul(
                            c3t[:], t["bd"][:, h * P:(h + 1) * P],
                            t["xld"][:, c0:c1], start=False, stop=True)
                        nc.vector.tensor_copy(
                            t["out"][:, h * C + c0: h * C + c1], c3t[:])

            def emit_stores(t):
                half = HT * C // 2
                nc.sync.dma_start(out_d[:, :half], t["out"][:, :half])
                nc.scalar.dma_start(out_d[:, half:], t["out"][:, half:])

            hoisted = None
            if body == "compute":
                hoisted = make_tiles()
                emit_loads(hoisted)

            loop_ctx = None
            if loop_reps > 0:
                loop_ctx = tc.For_i(
                    0, loop_reps, 1,
                    hint_engines=(mybir.EngineType.PE, mybir.EngineType.DVE,
                                  mybir.EngineType.Activation,
                                  mybir.EngineType.SP))
                loop_ctx.__enter__()

            for _rep in range(repeat):
                t = hoisted if hoisted is not None else make_tiles()
                if body == "full":
                    emit_loads(t)
                    emit_compute(t)
                elif body == "dma":
                    emit_loads(t)
                elif body == "compute":
                    emit_compute(t)
                if body != "dma":
                    emit_stores(t)

            if loop_ctx is not None:
                loop_ctx.__exit__(None, None, None)

    nc.compile()
    return nc


def _get_nc(C, mode=None, repeat=1, loop_reps=0, body="full"):
    key = (C, mode, repeat, loop_reps, body)
    if key not in _CACHE:
        _CACHE[key] = _build(C, mode, repeat, loop_reps, body)
    return _CACHE[key]


def kernel(hidden_states, topk_weights, w13, w2, gate_up_lora_a,
           gate_up_lora_b, down_lora_a, down_lora_b, scalings,
           topk_ids, lora_indices, mode=None):
    in_maps, idx_per, tok, C = _prep_in_maps(
        hidden_states, topk_weights, w13, w2, gate_up_lora_a,
        gate_up_lora_b, down_lora_a, down_lora_b, scalings,
        topk_ids, lora_indices, mode=mode)
    nc = _get_nc(C, mode)
    res = run_bass_kernel_spmd(nc, in_maps, list(range(E)))
    out = _combine(res.results, idx_per, tok, C)
    return out.astype(np.asarray(hidden_states).dtype)



# revision 9
# speedup vs baseline: 2532.5841x; 2532.5841x over previous
"""Fused MoE + per-expert LoRA, expert-parallel across 8 TRN2 NeuronCores.

Strategy (sharding hint: expert-parallel):
  - Host dispatches the T*K routed (token, expert) pairs: core e gets the
    tokens routed to expert e, gathered + transposed to [H, C] (C = padded
    max per-expert count), plus expert e's w13/w2 and the 4 LoRA adapters'
    A/B for expert e, all pre-transposed on host into the exact SBUF
    layouts the kernel consumes (contiguous DMAs).
  - LoRA is fused into the base GEMMs: the 4 adapters' A matrices are
    concatenated to [64, H] so one extra K=128-tile matmul chain produces
    all mid-products; a [64, C] mask (scaling[l] where the pair's adapter
    == l, else 0) selects the right adapter per column; the 4 B matrices
    concatenated to [64, N] accumulate into the same PSUM tile as the base
    GEMM (one extra K=64 matmul per output tile).
  - Routing weight is folded into the activation (act = silu(gate)*up*w),
    which makes both the base down-GEMM and the down-LoRA delta carry it.
  - Host scatter-adds each core's [H, C] output back over the top_k axis.
  - Matmuls run in float32r (full-rate fp32 streaming, ~1e-4 rel err).
"""

import numpy as np
from contextlib import ExitStack

import concourse.bass as bass
import concourse.tile as tile
from concourse import bacc, mybir
from concourse.bass_utils import run_bass_kernel_spmd

T, H, I, E, K, L, R = 1024, 1024, 1024, 8, 2, 4, 16
N = 2 * I
P = 128
KH = H // P    # k-tiles over H (gate_up contraction)
KI = I // P    # k-tiles over I (down contraction)
NT = N // P    # n-tiles of gate_up output
HT = H // P    # h-tiles of down output
LR = L * R     # concatenated lora rank

W13_GRP = 4              # n-tile slots per DMA group
GS13 = KH * W13_GRP * P  # free-dim span of one w13 group in SBUF
W2_GRP = 4
GS2 = KI * W2_GRP * P
# w13 SBUF slot s holds n-tile W13_PERM[s]: gate/up tiles interleaved so the
# compute loop consumes slots sequentially (pair j = slots 2j, 2j+1)
W13_PERM = [t for j in range(NT // 2) for t in (j, j + NT // 2)]

_CACHE: dict = {}

# f16 halves HBM traffic vs f32 (w13+w2 dominate); matmul streams at the
# same 1 col/cycle either way, so this moves the kernel from DMA-bound
# (~46us of weight DMA) to PE-bound (~29us). randn-scale data sits well
# inside f16 range; rel err ~1e-3 << the 2e-2 gate.
MODE = "f16"


def _round_up(x, m):
    return ((x + m - 1) // m) * m


def _np_dt(mode):
    if mode == "bf16":
        import ml_dtypes
        return np.dtype(ml_dtypes.bfloat16)
    if mode == "f16":
        return np.dtype(np.float16)
    return np.dtype(np.float32)


def _mm_dt(mode):
    return {"f32": mybir.dt.float32,
            "f32r": mybir.dt.float32r,
            "bf16": mybir.dt.bfloat16,
            "f16": mybir.dt.float16}[mode]


def _prep_in_maps(hidden_states, topk_weights, w13, w2, gate_up_lora_a,
                  gate_up_lora_b, down_lora_a, down_lora_b, scalings,
                  topk_ids, lora_indices, mode=None):
    """Host-side dispatch: returns (in_maps, idx_per_expert, tok, C)."""
    mode = mode or MODE
    ndt = _np_dt(mode)
    hidden_states = np.asarray(hidden_states, dtype=np.float32)
    topk_weights = np.asarray(topk_weights, dtype=np.float32)
    w13 = np.asarray(w13, dtype=np.float32)
    w2 = np.asarray(w2, dtype=np.float32)
    gua = np.asarray(gate_up_lora_a, dtype=np.float32)
    gub = np.asarray(gate_up_lora_b, dtype=np.float32)
    dla = np.asarray(down_lora_a, dtype=np.float32)
    dlb = np.asarray(down_lora_b, dtype=np.float32)
    scalings = np.asarray(scalings, dtype=np.float32)
    topk_ids = np.asarray(topk_ids)
    lora_indices = np.asarray(lora_indices)

    e_flat = topk_ids.reshape(-1).astype(np.int64)
    l_flat = np.repeat(lora_indices, K).astype(np.int64)
    w_flat = topk_weights.reshape(-1).astype(np.float32)
    tok = np.arange(T * K) // K

    idx_per = [np.nonzero(e_flat == e)[0] for e in range(E)]
    maxc = max(1, max(len(ix) for ix in idx_per))
    C = max(128, _round_up(maxc, 32))

    in_maps = []
    for e in range(E):
        ix = idx_per[e]
        cnt = len(ix)

        xg = np.zeros((C, H), np.float32)
        xg[:cnt] = hidden_states[tok[ix]]
        xt = np.ascontiguousarray(
            xg.T.reshape(KH, P, C).transpose(1, 0, 2).reshape(P, KH * C)).astype(ndt)

        w13t = w13[e].T  # [H, N]
        # slot order interleaves gate/up n-tiles: slot 2j = gate j, 2j+1 = up j
        w13_tiles = w13t.reshape(KH, P, NT, P)[:, :, W13_PERM]
        w13_hbm = np.ascontiguousarray(
            w13_tiles.reshape(KH, P, NT // W13_GRP, W13_GRP * P)
            .transpose(1, 2, 0, 3).reshape(P, KH * N)).astype(ndt)

        w2t = w2[e].T  # [I, H]
        w2_hbm = np.ascontiguousarray(
            w2t.reshape(KI, P, H // (W2_GRP * P), W2_GRP * P)
            .transpose(1, 2, 0, 3).reshape(P, KI * H)).astype(ndt)

        ag = np.ascontiguousarray(
            gua[:, e].reshape(LR, H).T.reshape(KH, P, LR)
            .transpose(1, 0, 2).reshape(P, KH * LR)).astype(ndt)
        ad = np.ascontiguousarray(
            dla[:, e].reshape(LR, I).T.reshape(KI, P, LR)
            .transpose(1, 0, 2).reshape(P, KI * LR)).astype(ndt)
        bg = np.ascontiguousarray(
            gub[:, e].transpose(0, 2, 1).reshape(LR, N)).astype(ndt)
        bd = np.ascontiguousarray(
            dlb[:, e].transpose(0, 2, 1).reshape(LR, H)).astype(ndt)

        msk = np.zeros((LR, C), np.float32)
        if cnt:
            lv = l_flat[ix]
            m_small = (lv[None, :] == np.arange(L)[:, None]) * scalings[:, None]
            msk[:, :cnt] = np.repeat(m_small.astype(np.float32), R, axis=0)

        wv = np.zeros((P, C), np.float32)
        if cnt:
            wv[:, :cnt] = w_flat[ix][None, :]

        in_maps.append({
            "xt": xt, "w13t": w13_hbm, "w2t": w2_hbm,
            "agt": ag, "adt": ad, "bgt": bg, "bdt": bd,
            "msk": msk, "wv": wv,
        })
    return in_maps, idx_per, tok, C


def _combine(results, idx_per, tok, C):
    out = np.zeros((T, H), np.float32)
    for e in range(E):
        ix = idx_per[e]
        cnt = len(ix)
        if cnt == 0:
            continue
        outt = results[e]["outt"].reshape(P, HT, C).transpose(1, 0, 2).reshape(H, C)
        np.add.at(out, tok[ix], outt[:, :cnt].T)
    return out


def _build(C, mode=None, repeat=1, loop_reps=0, body="full"):
    """Trace + compile the per-core bass program for padded count C.

    loop_reps > 0 wraps the body in a device-side For_i loop (timing only).
    body: "full" | "dma" (loads/stores only, no compute) | "compute"
    (loads hoisted out of the timing loop) - diagnostics only.
    """
    mode = mode or MODE
    f32 = mybir.dt.float32
    mdt = _mm_dt(mode)
    nc = bacc.Bacc("TRN2", target_bir_lowering=False, debug=False, num_devices=E)

    xt_d = nc.declare_dram_parameter("xt", [P, KH * C], mdt, isOutput=False)
    w13_d = nc.declare_dram_parameter("w13t", [P, KH * N], mdt, isOutput=False)
    w2_d = nc.declare_dram_parameter("w2t", [P, KI * H], mdt, isOutput=False)
    ag_d = nc.declare_dram_parameter("agt", [P, KH * LR], mdt, isOutput=False)
    ad_d = nc.declare_dram_parameter("adt", [P, KI * LR], mdt, isOutput=False)
    bg_d = nc.declare_dram_parameter("bgt", [LR, N], mdt, isOutput=False)
    bd_d = nc.declare_dram_parameter("bdt", [LR, H], mdt, isOutput=False)
    msk_d = nc.declare_dram_parameter("msk", [LR, C], f32, isOutput=False)
    wv_d = nc.declare_dram_parameter("wv", [P, C], f32, isOutput=False)
    odt = f32 if mybir.dt.size(mdt) == 4 else mdt
    out_d = nc.declare_dram_parameter("outt", [P, HT * C], odt, isOutput=True)

    # column blocks of at most 512 (PSUM free-dim limit for fp32)
    n_blk = (C + 511) // 512
    step = (C + n_blk - 1) // n_blk
    blks = [(b * step, min(C, (b + 1) * step)) for b in range(n_blk)]

    silu_fn = mybir.ActivationFunctionType.Silu

    with tile.TileContext(nc) as tc:
        with ExitStack() as ctx:
            static = ctx.enter_context(tc.tile_pool(name="static", bufs=1))
            work = ctx.enter_context(tc.tile_pool(name="work", bufs=4))
            ptmp = ctx.enter_context(tc.tile_pool(name="ptmp", bufs=1, space="PSUM"))
            pc1 = ctx.enter_context(tc.tile_pool(name="pc1", bufs=2, space="PSUM"))
            pc3 = ctx.enter_context(tc.tile_pool(name="pc3", bufs=3, space="PSUM"))

            # double-buffer input tiles for 2-byte modes so a loop/repeat
            # iteration's DMAs overlap the previous iteration's compute
            # (f32 tiles are too big to double-buffer in SBUF)
            sb = 2 if mybir.dt.size(mdt) == 2 else 1

            def make_tiles():
                t = {}
                t["xt"] = static.tile([P, KH * C], mdt, tag="xt", name="xt_sb", bufs=sb)
                t["w13"] = static.tile([P, KH * N], mdt, tag="w13", name="w13_sb", bufs=sb)
                t["w2"] = static.tile([P, KI * H], mdt, tag="w2", name="w2_sb", bufs=sb)
                t["ag"] = static.tile([P, KH * LR], mdt, tag="ag", name="ag_sb", bufs=sb)
                t["ad"] = static.tile([P, KI * LR], mdt, tag="ad", name="ad_sb", bufs=sb)
                t["bg"] = static.tile([LR, N], mdt, tag="bg", name="bg_sb", bufs=sb)
                t["bd"] = static.tile([LR, H], mdt, tag="bd", name="bd_sb", bufs=sb)
                t["msk"] = static.tile([LR, C], f32, tag="msk", name="msk_sb", bufs=sb)
                t["wv"] = static.tile([P, C], f32, tag="wv", name="wv_sb", bufs=sb)
                t["act"] = static.tile([P, KI * C], mdt, tag="act", name="act_sb")
                t["out"] = static.tile([P, HT * C], odt, tag="out", name="out_sb")
                t["xlg"] = static.tile([LR, C], mdt, tag="xlg", name="xlg_sb")
                t["xld"] = static.tile([LR, C], mdt, tag="xld", name="xld_sb")
                return t

            def emit_loads(t):
                xh = (KH // 2) * C
                nc.sync.dma_start(t["xt"][:, :xh], xt_d[:, :xh])
                nc.scalar.dma_start(t["ag"][:], ag_d[:])
                nc.sync.dma_start(t["xt"][:, xh:], xt_d[:, xh:])
                nc.scalar.dma_start(t["msk"][:], msk_d[:])
                nc.scalar.dma_start(t["bg"][:], bg_d[:])
                nc.scalar.dma_start(t["wv"][:], wv_d[:])
                # w13 groups alternate rings, in consumption order
                for g in range(N // (W13_GRP * P)):
                    eng = nc.sync if g % 2 == 0 else nc.scalar
                    eng.dma_start(t["w13"][:, g * GS13:(g + 1) * GS13],
                                  w13_d[:, g * GS13:(g + 1) * GS13])
                # phase-B tensors after w13
                nc.sync.dma_start(t["ad"][:], ad_d[:])
                nc.scalar.dma_start(t["bd"][:], bd_d[:])
                for g in range(H // (W2_GRP * P)):
                    eng = nc.sync if g % 2 == 0 else nc.scalar
                    eng.dma_start(t["w2"][:, g * GS2:(g + 1) * GS2],
                                  w2_d[:, g * GS2:(g + 1) * GS2])

            def emit_compute(t):
                for (c0, c1) in blks:
                    cw = c1 - c0

                    def xts(kt):
                        return t["xt"][:, kt * C + c0: kt * C + c1]

                    def acts(kt):
                        return t["act"][:, kt * C + c0: kt * C + c1]

                    # gate_up lora mid-product
                    tmp_g = ptmp.tile([LR, cw], f32, tag="tmp")
                    for kt in range(KH):
                        nc.tensor.matmul(
                            tmp_g[:], t["ag"][:, kt * LR:(kt + 1) * LR], xts(kt),
                            start=(kt == 0), stop=(kt == KH - 1))
                    nc.vector.tensor_mul(t["xlg"][:, c0:c1], tmp_g[:],
                                         t["msk"][:, c0:c1])

                    # gate_up GEMM + lora, act = silu(gate)*up*w
                    for j in range(KI):
                        pair = []
                        for jn in (j, j + KI):
                            c1t = pc1.tile([P, cw], f32,
                                           tag="c1g" if jn == j else "c1u")
                            slot = 2 * j + (0 if jn == j else 1)
                            g, jj = divmod(slot, W13_GRP)
                            for kt in range(KH):
                                off = g * GS13 + kt * (W13_GRP * P) + jj * P
                                nc.tensor.matmul(
                                    c1t[:], t["w13"][:, off:off + P], xts(kt),
                                    start=(kt == 0), stop=False)
                            nc.tensor.matmul(
                                c1t[:], t["bg"][:, jn * P:(jn + 1) * P],
                                t["xlg"][:, c0:c1], start=False, stop=True)
                            pair.append(c1t)
                        c1g, c1u = pair
                        silu_t = work.tile([P, cw], f32, tag="silu")
                        up_w = work.tile([P, cw], f32, tag="upw")
                        nc.scalar.activation(silu_t[:], c1g[:], silu_fn)
                        nc.vector.tensor_mul(up_w[:], c1u[:], t["wv"][:, c0:c1])
                        nc.vector.tensor_mul(acts(j), silu_t[:], up_w[:])

                    # down lora mid-product
                    tmp_d = ptmp.tile([LR, cw], f32, tag="tmp")
                    for kt in range(KI):
                        nc.tensor.matmul(
                            tmp_d[:], t["ad"][:, kt * LR:(kt + 1) * LR], acts(kt),
                            start=(kt == 0), stop=(kt == KI - 1))
                    nc.vector.tensor_mul(t["xld"][:, c0:c1], tmp_d[:],
                                         t["msk"][:, c0:c1])

                    # down GEMM + lora
                    for h in range(HT):
                        c3t = pc3.tile([P, cw], f32, tag="c3")
                        g, jj = divmod(h, W2_GRP)
                        for kt in range(KI):
                            off = g * GS2 + kt * (W2_GRP * P) + jj * P
                            nc.tensor.matmul(
                                c3t[:], t["w2"][:, off:off + P], acts(kt),
                                start=(kt == 0), stop=False)
                        nc.tensor.matmul(
                            c3t[:], t["bd"][:, h * P:(h + 1) * P],
                            t["xld"][:, c0:c1], start=False, stop=True)
                        if h % 2 == 0:
                            nc.vector.tensor_copy(
                                t["out"][:, h * C + c0: h * C + c1], c3t[:])
                        else:
                            nc.scalar.activation(
                                t["out"][:, h * C + c0: h * C + c1], c3t[:],
                                mybir.ActivationFunctionType.Copy)
                        deng = nc.sync if h % 2 == 0 else nc.scalar
                        deng.dma_start(out_d[:, h * C + c0: h * C + c1],
                                       t["out"][:, h * C + c0: h * C + c1])

            def emit_stores(t):
                pass

            hoisted = None
            if body == "compute":
                hoisted = make_tiles()
                emit_loads(hoisted)

            loop_ctx = None
            if loop_reps > 0:
                loop_ctx = tc.For_i(
                    0, loop_reps, 1,
                    hint_engines=(mybir.EngineType.PE, mybir.EngineType.DVE,
                                  mybir.EngineType.Activation,
                                  mybir.EngineType.SP))
                loop_ctx.__enter__()

            for _rep in range(repeat):
                t = hoisted if hoisted is not None else make_tiles()
                if body == "full":
                    emit_loads(t)
                    emit_compute(t)
                elif body == "dma":
                    emit_loads(t)
                elif body == "compute":
                    emit_compute(t)
                if body != "dma":
                    emit_stores(t)

            if loop_ctx is not None:
                loop_ctx.__exit__(None, None, None)

    nc.compile()
    return nc


def _get_nc(C, mode=None, repeat=1, loop_reps=0, body="full"):
    mode = mode or MODE
    key = (C, mode, repeat, loop_reps, body)
    if key not in _CACHE:
        _CACHE[key] = _build(C, mode, repeat, loop_reps, body)
    return _CACHE[key]


def kernel(hidden_states, topk_weights, w13, w2, gate_up_lora_a,
           gate_up_lora_b, down_lora_a, down_lora_b, scalings,
           topk_ids, lora_indices, mode=None):
    in_maps, idx_per, tok, C = _prep_in_maps(
        hidden_states, topk_weights, w13, w2, gate_up_lora_a,
        gate_up_lora_b, down_lora_a, down_lora_b, scalings,
        topk_ids, lora_indices, mode=mode)
    nc = _get_nc(C, mode)
    res = run_bass_kernel_spmd(nc, in_maps, list(range(E)))
    out = _combine(res.results, idx_per, tok, C)
    return out.astype(np.asarray(hidden_states).dtype)

